# revision 2
# baseline (speedup 1.0000x reference)
"""Confusion-matrix kernel for Trainium2 (8 NeuronCores, data-parallel over batch).

Per batch b (one per core):
    pred[n]  = argmax_c input[b, c, n]            (n = pixel, N = H*W)
    raw[i, j] = sum_n target[b, i, n] * (pred[n] == j)
Host: cm_b = raw / rowsum(raw); out = mean_b cm_b   (row-normalization absorbs
multi-hot double counting from value ties after quantization).

Device layout: class-outer [P, C, K] tiles (partition p holds C*K values:
C class-rows of K consecutive pixels).  All DVE ops then have innermost
step-1 16-bit APs -> 2x perf mode:
  - max over classes: 6-level tensor_tensor max tree (not 1x tensor_reduce)
  - one-hot: is_ge against the max broadcast over the middle (class) axis
Matmul per pixel column k: lhsT = y[:, :, k] (fp8 weights, [128 pix, 21]),
rhs = h[:, :, k] ([128 pix, 21]), accumulated into a [21, 21] f32 PSUM tile.

Raw-bass pipeline (double buffered, NT tiles):
    SP   : x-load even t  (gated on sv >= t-1 : is_ge(t-2) freed x slot)
    Pool : x-load odd  t  (same gating; separate queue keeps DMA fed)
    ACT  : y-load t       (gated on sp >= t-1 : matmuls(t-2) freed y slot)
    DVE  : max-tree -> is_ge(t)  (gated on x arrival; h slot on sp >= t-1)
    PE   : K matmuls(t)          (gated on sv >= t+1, y arrival)
"""

from contextlib import ExitStack

import ml_dtypes
import numpy as np

import concourse.bass as bass
import concourse.mybir as mybir
from concourse.bass_utils import run_bass_kernel_spmd

B, C, H, W = 8, 21, 512, 512
N = H * W            # 262144 pixels per batch
P = 128              # SBUF partitions
K = 512              # pixels per partition per tile
NT = N // (P * K)    # 4 outer tiles per core
N_CORES = 8

X_NP_DT = np.float16
X_BIR_DT = mybir.dt.float16
Y_NP_DT = ml_dtypes.float8_e4m3
Y_BIR_DT = mybir.dt.float8e4

_CACHED_NC = None


def build_nc():
    nc = bass.Bass()
    x = nc.declare_dram_parameter("x", [NT, P, C * K], X_BIR_DT, isOutput=False)
    y = nc.declare_dram_parameter("y", [NT, P, C * K], Y_BIR_DT, isOutput=False)
    out = nc.declare_dram_parameter("out", [C, C], mybir.dt.float32, isOutput=True)

    with ExitStack() as ctx:
        xs = [
            ctx.enter_context(nc.sbuf_tensor(f"xsb{i}", [P, C * K], X_BIR_DT))
            for i in range(2)
        ]
        ys = [
            ctx.enter_context(nc.sbuf_tensor(f"ysb{i}", [P, C * K], Y_BIR_DT))
            for i in range(2)
        ]
        hs = [
            ctx.enter_context(nc.sbuf_tensor(f"hsb{i}", [P, C * K], mybir.dt.float16))
            for i in range(2)
        ]
        ma = ctx.enter_context(nc.sbuf_tensor("ma", [P, 10 * K], mybir.dt.float16))
        mb = ctx.enter_context(nc.sbuf_tensor("mb", [P, 5 * K], mybir.dt.float16))
        mc = ctx.enter_context(nc.sbuf_tensor("mc", [P, 2 * K], mybir.dt.float16))
        md = ctx.enter_context(nc.sbuf_tensor("md", [P, K], mybir.dt.float16))
        me = ctx.enter_context(nc.sbuf_tensor("me", [P, K], mybir.dt.float16))
        mm = ctx.enter_context(nc.sbuf_tensor("mm", [P, K], mybir.dt.float16))
        ot = ctx.enter_context(nc.sbuf_tensor("otsb", [C, C], mybir.dt.float32))
        cm_psum = ctx.enter_context(nc.psum_tensor("cmps", [C, C], mybir.dt.float32))

        block = ctx.enter_context(nc.Block())
        sxs = [ctx.enter_context(nc.semaphore(f"sx{i}")) for i in range(2)]
        sys_ = [ctx.enter_context(nc.semaphore(f"sy{i}")) for i in range(2)]
        sv = ctx.enter_context(nc.semaphore("sv"))
        sp = ctx.enter_context(nc.semaphore("sp"))
        so = ctx.enter_context(nc.semaphore("so"))

        @block.sync
        def _(sync):
            for t in range(0, NT, 2):
                if t >= 2:
                    sync.wait_ge(sv, t - 1)
                sync.dma_start(out=xs[t % 2][:], in_=x[t]).then_inc(sxs[t % 2], 16)
            sync.wait_ge(sv, NT + 1)
            sync.dma_start(out=out[:], in_=ot[:]).then_inc(so, 16)
            sync.wait_ge(so, 16)

        @block.gpsimd
        def _(gp):
            for t in range(1, NT, 2):
                if t >= 2:
                    gp.wait_ge(sv, t - 1)
                gp.dma_start(out=xs[t % 2][:], in_=x[t]).then_inc(sxs[t % 2], 16)

        @block.scalar
        def _(scalar):
            for t in range(NT):
                if t >= 2:
                    scalar.wait_ge(sp, t - 1)
                scalar.dma_start(out=ys[t % 2][:], in_=y[t]).then_inc(sys_[t % 2], 16)

        @block.vector
        def _(vector):
            TT = nc.vector.tensor_tensor
            mx = mybir.AluOpType.max
            ma3 = ma[:].rearrange("p (c k) -> p c k", c=10)
            mb3 = mb[:].rearrange("p (c k) -> p c k", c=5)
            mc3 = mc[:].rearrange("p (c k) -> p c k", c=2)
            md3 = md[:].unsqueeze(1)
            me3 = me[:].unsqueeze(1)
            mm3 = mm[:].unsqueeze(1)
            for t in range(NT):
                x3 = xs[t % 2][:].rearrange("p (c k) -> p c k", c=C)
                h3 = hs[t % 2][:].rearrange("p (c k) -> p c k", c=C)
                vector.wait_ge(sxs[t % 2], 16 * (t // 2 + 1))
                TT(out=ma3, in0=x3[:, 0:10, :], in1=x3[:, 10:20, :], op=mx)
                TT(out=mb3, in0=ma3[:, 0:5, :], in1=ma3[:, 5:10, :], op=mx)
                TT(out=mc3, in0=mb3[:, 0:2, :], in1=mb3[:, 2:4, :], op=mx)
                TT(out=md3, in0=mc3[:, 0:1, :], in1=mc3[:, 1:2, :], op=mx)
                TT(out=me3, in0=md3, in1=mb3[:, 4:5, :], op=mx)
                TT(out=mm3, in0=me3, in1=x3[:, 20:21, :], op=mx)
                if t >= 2:
                    # h slot freed once matmuls(t-2) consumed it
                    vector.wait_ge(sp, t - 1)
                TT(
                    out=h3,
                    in0=x3,
                    in1=mm3.to_broadcast((P, C, K)),
                    op=mybir.AluOpType.is_ge,
                ).then_inc(sv, 1)  # sv = t + 1
            vector.wait_ge(sp, NT)
            nc.vector.tensor_copy(ot[:], cm_psum[:]).then_inc(sv, 1)  # sv = NT + 1

        @block.tensor
        def _(tensor):
            for t in range(NT):
                y3 = ys[t % 2][:].rearrange("p (c k) -> p c k", c=C)
                h3 = hs[t % 2][:].rearrange("p (c k) -> p c k", c=C)
                tensor.wait_ge(sv, t + 1)     # is_ge(t) done
                tensor.wait_ge(sys_[t % 2], 16 * (t // 2 + 1))
                for k in range(K):
                    mmu = nc.tensor.matmul(
                        out=cm_psum[:],
                        lhsT=y3[:, :, k : k + 1],
                        rhs=h3[:, :, k : k + 1],
                        start=(t == 0 and k == 0),
                        stop=(t == NT - 1 and k == K - 1),
                    )
                mmu.then_inc(sp, 1)  # sp = t + 1

    return nc


def _get_nc():
    global _CACHED_NC
    if _CACHED_NC is None:
        _CACHED_NC = build_nc()
    return _CACHED_NC


def make_in_maps(input, target):
    inp = np.asarray(input, dtype=np.float32)
    tgt = np.asarray(target, dtype=np.float32)
    in_maps = []
    for b in range(B):
        # [C, N] -> [C, NT, P, K] -> [NT, P, C, K]: per (tile, partition) a
        # class-outer block of K consecutive pixels. No host transpose of the
        # pixel axis needed - blocks are contiguous runs of the input rows.
        xb = inp[b].reshape(C, NT, P, K).transpose(1, 2, 0, 3)
        yb = tgt[b].reshape(C, NT, P, K).transpose(1, 2, 0, 3)
        in_maps.append(
            {
                "x": np.ascontiguousarray(xb).astype(X_NP_DT).reshape(NT, P, C * K),
                "y": np.ascontiguousarray(yb).astype(Y_NP_DT).reshape(NT, P, C * K),
            }
        )
    return in_maps


def postprocess(outs):
    acc = np.stack([np.asarray(o, dtype=np.float64) for o in outs])  # [B, C, C]
    cm = acc / (acc.sum(axis=2, keepdims=True) + 1e-30)
    return cm.mean(axis=0).astype(np.float32)


def kernel(input, target):
    nc = _get_nc()
    in_maps = make_in_maps(input, target)
    res = run_bass_kernel_spmd(nc, in_maps, list(range(N_CORES)))
    return postprocess([r["out"] for r in res.results])


# revision 8
# speedup vs baseline: 1.2207x; 1.2207x over previous
"""Confusion-matrix kernel for Trainium2 (8 NeuronCores, data-parallel over batch).

Per batch b (one per core):
    pred[n]  = argmax_c input[b, c, n]            (n = pixel, N = H*W)
    raw[i, j] = sum_n target[b, i, n] * (pred[n] == j)
Host: cm_b = raw / rowsum(raw); out = mean_b cm_b   (row-normalization absorbs
multi-hot double counting from value ties after quantization).

Device layout: class-outer [P, C, K] tiles (partition p holds C*K values:
C class-rows of K consecutive pixels).  All DVE ops then have innermost
step-1 16-bit APs -> 2x perf mode:
  - max over classes: 6-level tensor_tensor max tree (not 1x tensor_reduce)
  - one-hot: is_ge against the max broadcast over the middle (class) axis
Matmul per pixel column k: lhsT = y[k] (fp8 weights, [128 pix, 21], pixel-major
contiguous), rhs = h[:, :, k] ([128 pix, 21]), accumulated into PSUM. The PE
array is column-tiled 128x32: pixel column k runs on col-tile k%4, so four
ldweights+matmul lanes execute concurrently in disjoint 32-column groups of
the array, each accumulating its own [21, 21] PSUM block (partitions 32j..).
Host sums the four blocks.

Raw-bass pipeline (double buffered, NT tiles):
    SP   : x-load even t  (gated on sv >= t-1 : is_ge(t-2) freed x slot)
    Pool : x-load odd  t  (same gating; separate queue keeps DMA fed)
    ACT  : y-load t       (gated on sp >= t-1 : matmuls(t-2) freed y slot)
    DVE  : max-tree -> is_ge(t)  (gated on x arrival; h slot on sp >= t-1)
    PE   : K matmuls(t)          (gated on sv >= t+1, y arrival)
"""

from contextlib import ExitStack

import ml_dtypes
import numpy as np

import concourse.bass as bass
import concourse.mybir as mybir
from concourse.bass_utils import run_bass_kernel_spmd

B, C, H, W = 8, 21, 512, 512
N = H * W            # 262144 pixels per batch
P = 128              # SBUF partitions
K = 512              # pixels per partition per tile
NT = N // (P * K)    # 4 outer tiles per core
N_CORES = 8

X_NP_DT = np.float16
X_BIR_DT = mybir.dt.float16
Y_NP_DT = ml_dtypes.float8_e4m3
Y_BIR_DT = mybir.dt.float8e4

_CACHED_NC = None


def build_nc():
    nc = bass.Bass()
    x = nc.declare_dram_parameter("x", [NT, P, C * K], X_BIR_DT, isOutput=False)
    y = nc.declare_dram_parameter("y", [NT, P, C * K], Y_BIR_DT, isOutput=False)
    out = nc.declare_dram_parameter("out", [P, C], mybir.dt.float32, isOutput=True)

    with ExitStack() as ctx:
        xs = [
            ctx.enter_context(nc.sbuf_tensor(f"xsb{i}", [P, C * K], X_BIR_DT))
            for i in range(2)
        ]
        ys = [
            ctx.enter_context(nc.sbuf_tensor(f"ysb{i}", [P, C * K], Y_BIR_DT))
            for i in range(2)
        ]
        hs = [
            ctx.enter_context(nc.sbuf_tensor(f"hsb{i}", [P, C * K], mybir.dt.float16))
            for i in range(2)
        ]
        ma = ctx.enter_context(nc.sbuf_tensor("ma", [P, 10 * K], mybir.dt.float16))
        mb = ctx.enter_context(nc.sbuf_tensor("mb", [P, 5 * K], mybir.dt.float16))
        mc = ctx.enter_context(nc.sbuf_tensor("mc", [P, 2 * K], mybir.dt.float16))
        md = ctx.enter_context(nc.sbuf_tensor("md", [P, K], mybir.dt.float16))
        me = ctx.enter_context(nc.sbuf_tensor("me", [P, K], mybir.dt.float16))
        mm = ctx.enter_context(nc.sbuf_tensor("mm", [P, K], mybir.dt.float16))
        ot = ctx.enter_context(nc.sbuf_tensor("otsb", [P, C], mybir.dt.float32))
        cm_psum = ctx.enter_context(nc.psum_tensor("cmps", [P, C], mybir.dt.float32))

        block = ctx.enter_context(nc.Block())
        sxs = [ctx.enter_context(nc.semaphore(f"sx{i}")) for i in range(2)]
        sys_ = [ctx.enter_context(nc.semaphore(f"sy{i}")) for i in range(2)]
        sv = ctx.enter_context(nc.semaphore("sv"))
        sp = ctx.enter_context(nc.semaphore("sp"))
        so = ctx.enter_context(nc.semaphore("so"))

        @block.sync
        def _(sync):
            for t in range(0, NT, 2):
                if t >= 2:
                    sync.wait_ge(sv, t - 1)
                sync.dma_start(out=xs[t % 2][:], in_=x[t]).then_inc(sxs[t % 2], 16)
            sync.wait_ge(sv, NT + 1)
            sync.dma_start(out=out[:], in_=ot[:]).then_inc(so, 16)
            sync.wait_ge(so, 16)

        @block.gpsimd
        def _(gp):
            for t in range(1, NT, 2):
                if t >= 2:
                    gp.wait_ge(sv, t - 1)
                gp.dma_start(out=xs[t % 2][:], in_=x[t]).then_inc(sxs[t % 2], 16)

        @block.scalar
        def _(scalar):
            for t in range(NT):
                if t >= 2:
                    scalar.wait_ge(sp, t - 1)
                scalar.dma_start(out=ys[t % 2][:], in_=y[t]).then_inc(sys_[t % 2], 16)

        @block.vector
        def _(vector):
            TT = nc.vector.tensor_tensor
            mx = mybir.AluOpType.max
            ma3 = ma[:].rearrange("p (c k) -> p c k", c=10)
            mb3 = mb[:].rearrange("p (c k) -> p c k", c=5)
            mc3 = mc[:].rearrange("p (c k) -> p c k", c=2)
            md3 = md[:].unsqueeze(1)
            me3 = me[:].unsqueeze(1)
            mm3 = mm[:].unsqueeze(1)
            for t in range(NT):
                x3 = xs[t % 2][:].rearrange("p (c k) -> p c k", c=C)
                h3 = hs[t % 2][:].rearrange("p (c k) -> p c k", c=C)
                vector.wait_ge(sxs[t % 2], 16 * (t // 2 + 1))
                TT(out=ma3, in0=x3[:, 0:10, :], in1=x3[:, 10:20, :], op=mx)
                TT(out=mb3, in0=ma3[:, 0:5, :], in1=ma3[:, 5:10, :], op=mx)
                TT(out=mc3, in0=mb3[:, 0:2, :], in1=mb3[:, 2:4, :], op=mx)
                TT(out=md3, in0=mc3[:, 0:1, :], in1=mc3[:, 1:2, :], op=mx)
                TT(out=me3, in0=md3, in1=mb3[:, 4:5, :], op=mx)
                TT(out=mm3, in0=me3, in1=x3[:, 20:21, :], op=mx)
                if t >= 2:
                    # h slot freed once matmuls(t-2) consumed it
                    vector.wait_ge(sp, t - 1)
                TT(
                    out=h3,
                    in0=x3,
                    in1=mm3.to_broadcast((P, C, K)),
                    op=mybir.AluOpType.is_ge,
                ).then_inc(sv, 1)  # sv = t + 1
            vector.wait_ge(sp, NT)
            nc.vector.tensor_copy(ot[:], cm_psum[:]).then_inc(sv, 1)  # sv = NT + 1

        @block.tensor
        def _(tensor):
            for t in range(NT):
                yt = ys[t % 2][:]
                h3 = hs[t % 2][:].rearrange("p (c k) -> p c k", c=C)
                tensor.wait_ge(sv, t + 1)     # is_ge(t) done
                tensor.wait_ge(sys_[t % 2], 16 * (t // 2 + 1))
                for k in range(K):
                    j = k % 4  # column tile lane
                    mmu = nc.tensor.matmul(
                        out=cm_psum[:][32 * j : 32 * j + C, :],
                        lhsT=yt[:, k * C : (k + 1) * C],
                        rhs=h3[:, :, k : k + 1],
                        start=(t == 0 and k == j),
                        stop=(t == NT - 1 and k == K - 4 + j),
                        tile_position=(0, 32 * j),
                        skip_group_check=True,
                    )
                mmu.then_inc(sp, 1)  # sp = t + 1

    return nc


def _get_nc():
    global _CACHED_NC
    if _CACHED_NC is None:
        _CACHED_NC = build_nc()
    return _CACHED_NC


def make_in_maps(input, target):
    inp = np.asarray(input, dtype=np.float32)
    tgt = np.asarray(target, dtype=np.float32)
    in_maps = []
    for b in range(B):
        # [C, N] -> [C, NT, P, K] -> [NT, P, C, K]: per (tile, partition) a
        # class-outer block of K consecutive pixels. No host transpose of the
        # pixel axis needed - blocks are contiguous runs of the input rows.
        xb = inp[b].reshape(C, NT, P, K).transpose(1, 2, 0, 3)
        # y pixel-major: matmul weights y[k] are 21 contiguous bytes/partition
        yb = tgt[b].reshape(C, NT, P, K).transpose(1, 2, 3, 0)
        in_maps.append(
            {
                "x": np.ascontiguousarray(xb).astype(X_NP_DT).reshape(NT, P, C * K),
                "y": np.ascontiguousarray(yb).astype(Y_NP_DT).reshape(NT, P, C * K),
            }
        )
    return in_maps


def postprocess(outs):
    acc = np.stack([np.asarray(o, dtype=np.float64) for o in outs])  # [B, P, C]
    # sum the four column-tile lanes' [C, C] blocks (PSUM partitions 32j..)
    raw = sum(acc[:, 32 * j : 32 * j + C, :] for j in range(4))
    cm = raw / (raw.sum(axis=2, keepdims=True) + 1e-30)
    return cm.mean(axis=0).astype(np.float32)


def kernel(input, target):
    nc = _get_nc()
    in_maps = make_in_maps(input, target)
    res = run_bass_kernel_spmd(nc, in_maps, list(range(N_CORES)))
    return postprocess([r["out"] for r in res.results])


# revision 12
# speedup vs baseline: 1.3233x; 1.0840x over previous
"""Confusion-matrix kernel for Trainium2 (8 NeuronCores, data-parallel over batch).

Per batch b (one per core):
    pred[n]  = argmax_c input[b, c, n]            (n = pixel, N = H*W)
    raw[i, j] = sum_n target[b, i, n] * (pred[n] == j)
Host: cm_b = raw / rowsum(raw); out = mean_b cm_b   (row-normalization absorbs
multi-hot double counting from value ties after quantization).

Device layout: class-outer [P, C, K] tiles (partition p holds C class-rows of
K consecutive pixels).  All DVE ops then have innermost step-1 16-bit APs ->
2x perf mode:
  - max over classes: 6-level tensor_tensor max tree (not 1x tensor_reduce)
  - one-hot: is_ge against the max broadcast over the middle (class) axis

Matmul: 6 pixels per instruction.  lhsT = packed y for 6 pixels ([128 pix,
128 cols], cols a*21+i = y[pixel 6q+a, class i], 2 zero pad cols) - a full
128-column fp8 weight load triggers Fast Weight Load.  rhs = h for the same
6 pixels ([128 pix, 6, 21] view of the class-outer h tile, free dims
(pixel-within-pack outer, class inner)).  out accumulates [128, 126] in PSUM;
block (a, a) on the diagonal is the [21, 21] confusion matrix contribution of
pack position a.  Host sums the 6 diagonal blocks.  The last pack of each
tile overlaps the previous 6 pixels with zeroed weights so no rhs index goes
out of range.

Raw-bass pipeline (double buffered, NT tiles):
    SP   : x0, x2 loads; out store   (x2 gated on sv >= 1: is_ge(0) freed x0)
    Pool : x1, x3 loads (x1 gated on x0 arrival - keeps tile-0 latency low);
           gpsimd perf probe (timing only)
    ACT  : y loads (y0 gated on x0 arrival; y2/y3 on sp: matmuls freed slot)
    DVE  : max-tree -> is_ge(t)  (gated on x arrival; h slot on sp >= t-1)
    PE   : QP pack-matmuls(t)    (gated on sv >= t+1, y arrival)
"""

from contextlib import ExitStack

import ml_dtypes
import numpy as np

import concourse.bass as bass
import concourse.mybir as mybir
from concourse.bass_utils import run_bass_kernel_spmd

B, C, H, W = 8, 21, 512, 512
N = H * W            # 262144 pixels per batch
P = 128              # SBUF partitions
K = 512              # pixels per partition per tile
NT = N // (P * K)    # 4 outer tiles per core
N_CORES = 8

PACK = 6             # pixels per matmul (6*21 = 126 <= 128 weight cols)
MCOL = PACK * C      # 126 matmul columns
QP = (K + PACK - 1) // PACK   # 86 packs per tile (last one overlaps)

X_NP_DT = np.float16
X_BIR_DT = mybir.dt.float16
Y_NP_DT = ml_dtypes.float8_e4m3
Y_BIR_DT = mybir.dt.float8e4

_CACHED_NC = None


def build_nc():
    nc = bass.Bass()
    x = nc.declare_dram_parameter("x", [NT, P, C * K], X_BIR_DT, isOutput=False)
    y = nc.declare_dram_parameter("y", [NT, P, QP * P], Y_BIR_DT, isOutput=False)
    out = nc.declare_dram_parameter("out", [P, MCOL], mybir.dt.float32, isOutput=True)

    with ExitStack() as ctx:
        xs = [
            ctx.enter_context(nc.sbuf_tensor(f"xsb{i}", [P, C * K], X_BIR_DT))
            for i in range(2)
        ]
        ys = [
            ctx.enter_context(nc.sbuf_tensor(f"ysb{i}", [P, QP * P], Y_BIR_DT))
            for i in range(2)
        ]
        hs = [
            ctx.enter_context(nc.sbuf_tensor(f"hsb{i}", [P, C * K], mybir.dt.float16))
            for i in range(2)
        ]
        ma = ctx.enter_context(nc.sbuf_tensor("ma", [P, 10 * K], mybir.dt.float16))
        mb = ctx.enter_context(nc.sbuf_tensor("mb", [P, 5 * K], mybir.dt.float16))
        mc = ctx.enter_context(nc.sbuf_tensor("mc", [P, 2 * K], mybir.dt.float16))
        md = ctx.enter_context(nc.sbuf_tensor("md", [P, K], mybir.dt.float16))
        me = ctx.enter_context(nc.sbuf_tensor("me", [P, K], mybir.dt.float16))
        mm = ctx.enter_context(nc.sbuf_tensor("mm", [P, K], mybir.dt.float16))
        ot = ctx.enter_context(nc.sbuf_tensor("otsb", [P, MCOL], mybir.dt.float32))
        cm_psum = ctx.enter_context(nc.psum_tensor("cmps", [P, MCOL], mybir.dt.float32))

        block = ctx.enter_context(nc.Block())
        sxs = [ctx.enter_context(nc.semaphore(f"sx{i}")) for i in range(2)]
        sys_ = [ctx.enter_context(nc.semaphore(f"sy{i}")) for i in range(2)]
        sv = ctx.enter_context(nc.semaphore("sv"))
        sp = ctx.enter_context(nc.semaphore("sp"))
        so = ctx.enter_context(nc.semaphore("so"))

        @block.sync
        def _(sync):
            sync.dma_start(out=xs[0][:], in_=x[0]).then_inc(sxs[0], 16)
            if NT > 2:
                sync.wait_ge(sv, 1)   # is_ge(0) freed x slot 0
                sync.dma_start(out=xs[0][:], in_=x[2]).then_inc(sxs[0], 16)
            sync.wait_ge(sv, NT + 1)
            sync.dma_start(out=out[:], in_=ot[:]).then_inc(so, 16)
            sync.wait_ge(so, 16)

        @block.gpsimd
        def _(gp):
            # x0 first and alone: all DMA engines on the tile the critical
            # path starts with.  x1 only after x0 has landed.
            gp.wait_ge(sxs[0], 16)
            gp.dma_start(out=xs[1][:], in_=x[1]).then_inc(sxs[1], 16)
            if NT > 3:
                gp.wait_ge(sv, 2)     # is_ge(1) freed x slot 1
                gp.dma_start(out=xs[1][:], in_=x[3]).then_inc(sxs[1], 16)


        @block.scalar
        def _(scalar):
            for t in range(NT):
                if t == 0:
                    scalar.wait_ge(sxs[0], 16)   # let x0 use the engines alone
                if t >= 2:
                    scalar.wait_ge(sp, t - 1)    # matmuls(t-2) freed y slot
                scalar.dma_start(out=ys[t % 2][:], in_=y[t]).then_inc(sys_[t % 2], 16)

        @block.vector
        def _(vector):
            TT = nc.vector.tensor_tensor
            mx = mybir.AluOpType.max
            ma3 = ma[:].rearrange("p (c k) -> p c k", c=10)
            mb3 = mb[:].rearrange("p (c k) -> p c k", c=5)
            mc3 = mc[:].rearrange("p (c k) -> p c k", c=2)
            md3 = md[:].unsqueeze(1)
            me3 = me[:].unsqueeze(1)
            mm3 = mm[:].unsqueeze(1)
            for t in range(NT):
                x3 = xs[t % 2][:].rearrange("p (c k) -> p c k", c=C)
                h3 = hs[t % 2][:].rearrange("p (c k) -> p c k", c=C)
                vector.wait_ge(sxs[t % 2], 16 * (t // 2 + 1))
                TT(out=ma3, in0=x3[:, 0:10, :], in1=x3[:, 10:20, :], op=mx)
                TT(out=mb3, in0=ma3[:, 0:5, :], in1=ma3[:, 5:10, :], op=mx)
                TT(out=mc3, in0=mb3[:, 0:2, :], in1=mb3[:, 2:4, :], op=mx)
                TT(out=md3, in0=mc3[:, 0:1, :], in1=mc3[:, 1:2, :], op=mx)
                TT(out=me3, in0=md3, in1=mb3[:, 4:5, :], op=mx)
                TT(out=mm3, in0=me3, in1=x3[:, 20:21, :], op=mx)
                if t >= 2:
                    # h slot freed once matmuls(t-2) consumed it
                    vector.wait_ge(sp, t - 1)
                TT(
                    out=h3,
                    in0=x3,
                    in1=mm3.to_broadcast((P, C, K)),
                    op=mybir.AluOpType.is_ge,
                ).then_inc(sv, 1)  # sv = t + 1
            vector.wait_ge(sp, NT)
            nc.vector.tensor_copy(ot[:], cm_psum[:]).then_inc(sv, 1)  # sv = NT + 1

        @block.tensor
        def _(tensor):
            for t in range(NT):
                yt = ys[t % 2][:]
                # k-major view of the class-outer h tile: [P, K, C]
                hkc = hs[t % 2][:].rearrange("p (c k) -> p k c", c=C)
                tensor.wait_ge(sv, t + 1)     # is_ge(t) done
                tensor.wait_ge(sys_[t % 2], 16 * (t // 2 + 1))
                for q in range(QP):
                    k0 = min(q * PACK, K - PACK)  # last pack overlaps
                    mmu = nc.tensor.matmul(
                        out=cm_psum[:],
                        lhsT=yt[:, q * P : (q + 1) * P],
                        rhs=hkc[:, k0 : k0 + PACK, :],
                        start=(t == 0 and q == 0),
                        stop=(t == NT - 1 and q == QP - 1),
                    )
                mmu.then_inc(sp, 1)  # sp = t + 1

    return nc


def _get_nc():
    global _CACHED_NC
    if _CACHED_NC is None:
        _CACHED_NC = build_nc()
    return _CACHED_NC


def make_in_maps(input, target):
    inp = np.asarray(input, dtype=np.float32)
    tgt = np.asarray(target, dtype=np.float32)
    in_maps = []
    full = (K // PACK) * PACK          # 510 pixels in full packs
    for b in range(B):
        # x class-outer: [C, N] -> [C, NT, P, K] -> [NT, P, C, K]
        xb = inp[b].reshape(C, NT, P, K).transpose(1, 2, 0, 3)
        # y packed for 6-pixel matmul weights: [NT, P, K, C] pixel-major ->
        # packs of 6 pixels = 126 contiguous cols (+2 zero pad) per matmul
        yb = tgt[b].reshape(C, NT, P, K).transpose(1, 2, 3, 0)  # [NT, P, K, C]
        ypk = np.zeros((NT, P, QP, P), dtype=np.float32)
        ypk[:, :, : K // PACK, :MCOL] = yb[:, :, :full, :].reshape(
            NT, P, K // PACK, MCOL
        )
        # last pack reads pixels K-PACK..K; zero weights except the tail
        # pixels not covered by the full packs
        tail = K - full                 # 2
        ypk[:, :, QP - 1, MCOL - tail * C : MCOL] = yb[:, :, full:, :].reshape(
            NT, P, tail * C
        )
        in_maps.append(
            {
                "x": np.ascontiguousarray(xb).astype(X_NP_DT).reshape(NT, P, C * K),
                "y": np.ascontiguousarray(ypk).astype(Y_NP_DT).reshape(NT, P, QP * P),
            }
        )
    return in_maps


def postprocess(outs):
    acc = np.stack([np.asarray(o, dtype=np.float64) for o in outs])  # [B, P, MCOL]
    # sum the PACK diagonal [C, C] blocks of the [MCOL, MCOL] pack output
    raw = sum(acc[:, C * a : C * a + C, C * a : C * a + C] for a in range(PACK))
    cm = raw / (raw.sum(axis=2, keepdims=True) + 1e-30)
    return cm.mean(axis=0).astype(np.float32)


def kernel(input, target):
    nc = _get_nc()
    in_maps = make_in_maps(input, target)
    res = run_bass_kernel_spmd(nc, in_maps, list(range(N_CORES)))
    return postprocess([r["out"] for r in res.results])


# revision 14
# speedup vs baseline: 1.4172x; 1.0709x over previous
"""Confusion-matrix kernel for Trainium2 - fp8 inputs, non-uniform tiles.

Per batch b (one per core):
    pred[n]  = argmax_c input[b, c, n]
    raw[i, j] = sum_n target[b, i, n] * (pred[n] == j)
Host: cm_b = raw / rowsum(raw); out = mean_b cm_b.

x and y ship as fp8e4m3 (11 MB/core total).  The ACT engine upconverts each
x tile to fp16 (xf) so the DVE max-tree and is_ge run in 2x perf mode on
class-outer [P, C, Kt] tiles.  Tiles are non-uniform [128, 256, 512, 512,
512, 128] pixels/partition: small head tiles hide the ACT conversion latency
at pipeline fill, the small tail tile shrinks the serial is_ge+matmul tail.

Matmul: per pixel column k, on PE column-tile k%4 (128x32 column tiling ->
four concurrent ldweights+matmul lanes): lhsT = y[k] (fp8, 21 contiguous
bytes/partition, pixel-major), rhs = h3[:, :, k] (strided class-outer fp16),
each lane accumulating its own [21, 21] PSUM block at partitions 32j..
Host sums the 4 lane blocks, row-normalizes (absorbs fp8 argmax-tie double
counting), and means over batch.

Pipeline (2 buffers each for x8/xf/h/y):
    SP   : x loads even t; out store
    Pool : x loads odd t (first gated on x0 arrival)
    ACT  : y load issues + fp8->fp16 x conversions
    DVE  : max-tree -> is_ge(t)
    PE   : pack-matmuls(t)
"""

from contextlib import ExitStack

import ml_dtypes
import numpy as np

import concourse.bass as bass
import concourse.mybir as mybir
from concourse.bass_utils import run_bass_kernel_spmd

B, C, H, W = 8, 21, 512, 512
N = H * W
P = 128
KT = [128, 256, 512, 512, 512, 128]   # pixels/partition per tile
NT = len(KT)
assert sum(KT) == N // P
KOFF = [sum(KT[:t]) for t in range(NT)]   # flat pixel offsets
KMAX = max(KT)

N_CORES = 8

X_NP_DT = ml_dtypes.float8_e4m3
X_BIR_DT = mybir.dt.float8e4
Y_NP_DT = ml_dtypes.float8_e4m3
Y_BIR_DT = mybir.dt.float8e4

_CACHED_NC = None


def build_nc():
    nc = bass.Bass()
    # flat class-outer x / pixel-major y: per tile a contiguous [P, C*Kt] block
    x = nc.declare_dram_parameter("x", [P, C * (N // P)], X_BIR_DT, isOutput=False)
    y = nc.declare_dram_parameter("y", [P, C * (N // P)], Y_BIR_DT, isOutput=False)
    out = nc.declare_dram_parameter("out", [P, C], mybir.dt.float32, isOutput=True)

    with ExitStack() as ctx:
        x8 = [
            ctx.enter_context(nc.sbuf_tensor(f"x8b{i}", [P, C * KMAX], X_BIR_DT))
            for i in range(2)
        ]
        xf = [
            ctx.enter_context(nc.sbuf_tensor(f"xfb{i}", [P, C * KMAX], mybir.dt.float16))
            for i in range(2)
        ]
        ys = [
            ctx.enter_context(nc.sbuf_tensor(f"ysb{i}", [P, C * KMAX], Y_BIR_DT))
            for i in range(2)
        ]
        hs = [
            ctx.enter_context(nc.sbuf_tensor(f"hsb{i}", [P, C * KMAX], mybir.dt.float16))
            for i in range(2)
        ]
        ma = ctx.enter_context(nc.sbuf_tensor("ma", [P, 10 * KMAX], mybir.dt.float16))
        mb = ctx.enter_context(nc.sbuf_tensor("mb", [P, 5 * KMAX], mybir.dt.float16))
        mc = ctx.enter_context(nc.sbuf_tensor("mc", [P, 2 * KMAX], mybir.dt.float16))
        md = ctx.enter_context(nc.sbuf_tensor("md", [P, KMAX], mybir.dt.float16))
        me = ctx.enter_context(nc.sbuf_tensor("me", [P, KMAX], mybir.dt.float16))
        mm = ctx.enter_context(nc.sbuf_tensor("mm", [P, KMAX], mybir.dt.float16))
        ot = ctx.enter_context(nc.sbuf_tensor("otsb", [P, C], mybir.dt.float32))
        cm_psum = ctx.enter_context(nc.psum_tensor("cmps", [P, C], mybir.dt.float32))

        block = ctx.enter_context(nc.Block())
        sxs = [ctx.enter_context(nc.semaphore(f"sx{i}")) for i in range(2)]
        sys_ = [ctx.enter_context(nc.semaphore(f"sy{i}")) for i in range(2)]
        sf = ctx.enter_context(nc.semaphore("sf"))   # conversions done
        sv = ctx.enter_context(nc.semaphore("sv"))   # DVE tiles done
        sp = ctx.enter_context(nc.semaphore("sp"))   # PE tiles done
        so = ctx.enter_context(nc.semaphore("so"))

        def xin(t):
            return x[:, C * KOFF[t] : C * (KOFF[t] + KT[t])]

        def yin(t):
            return y[:, C * KOFF[t] : C * (KOFF[t] + KT[t])]

        # arrival count per buffer slot after tile t's DMA (inc 16 each)
        def arr(t):
            return 16 * (t // 2 + 1)

        @block.sync
        def _(sync):
            for t in range(0, NT, 2):
                if t >= 2:
                    sync.wait_ge(sf, t - 1)   # conv(t-2) freed x8 slot
                sync.dma_start(out=x8[0][:, : C * KT[t]], in_=xin(t)).then_inc(
                    sxs[0], 16
                )
            sync.wait_ge(sv, NT + 1)
            sync.dma_start(out=out[:], in_=ot[:]).then_inc(so, 16)
            sync.wait_ge(so, 16)

        def ydma(eng, t):
            if t >= 2:
                eng.wait_ge(sp, t - 1)    # matmuls(t-2) freed y slot
            eng.dma_start(out=ys[t % 2][:, : C * KT[t]], in_=yin(t)).then_inc(
                sys_[t % 2], 16
            )

        @block.gpsimd
        def _(gp):
            # x odd tiles + late y tiles.  Waits are ordered so no issue
            # blocks an earlier-needed one (sp/sf thresholds are increasing).
            gp.wait_ge(sxs[0], 16)        # x0 first and alone
            gp.dma_start(out=x8[1][:, : C * KT[1]], in_=xin(1)).then_inc(sxs[1], 16)
            gp.wait_ge(sf, 2)             # conv(1) freed x8 slot 1
            gp.dma_start(out=x8[1][:, : C * KT[3]], in_=xin(3)).then_inc(sxs[1], 16)
            ydma(gp, 2)
            gp.wait_ge(sf, 4)             # conv(3) freed x8 slot 1
            gp.dma_start(out=x8[1][:, : C * KT[5]], in_=xin(5)).then_inc(sxs[1], 16)
            for t in range(3, NT):
                ydma(gp, t)

        @block.scalar
        def _(scalar):
            # y0/y1 issues (quick), then the fp8->fp16 conversion chain
            scalar.wait_ge(sxs[0], 16)    # let x0 use the DMA engines alone
            ydma(scalar, 0)
            ydma(scalar, 1)
            for t in range(NT):
                scalar.wait_ge(sxs[t % 2], arr(t))
                if t >= 2:
                    scalar.wait_ge(sv, t - 1)  # DVE(t-2) done with xf slot
                nc.scalar.activation(
                    out=xf[t % 2][:, : C * KT[t]],
                    in_=x8[t % 2][:, : C * KT[t]],
                    func=mybir.ActivationFunctionType.Copy,
                ).then_inc(sf, 1)  # sf = t + 1

        @block.vector
        def _(vector):
            TT = nc.vector.tensor_tensor
            mx = mybir.AluOpType.max
            for t in range(NT):
                k = KT[t]
                x3 = xf[t % 2][:, : C * k].rearrange("p (c k) -> p c k", c=C)
                h3 = hs[t % 2][:, : C * k].rearrange("p (c k) -> p c k", c=C)
                ma3 = ma[:, : 10 * k].rearrange("p (c k) -> p c k", c=10)
                mb3 = mb[:, : 5 * k].rearrange("p (c k) -> p c k", c=5)
                mc3 = mc[:, : 2 * k].rearrange("p (c k) -> p c k", c=2)
                md3 = md[:, :k].unsqueeze(1)
                me3 = me[:, :k].unsqueeze(1)
                mm3 = mm[:, :k].unsqueeze(1)
                vector.wait_ge(sf, t + 1)
                TT(out=ma3, in0=x3[:, 0:10, :], in1=x3[:, 10:20, :], op=mx)
                TT(out=mb3, in0=ma3[:, 0:5, :], in1=ma3[:, 5:10, :], op=mx)
                TT(out=mc3, in0=mb3[:, 0:2, :], in1=mb3[:, 2:4, :], op=mx)
                TT(out=md3, in0=mc3[:, 0:1, :], in1=mc3[:, 1:2, :], op=mx)
                TT(out=me3, in0=md3, in1=mb3[:, 4:5, :], op=mx)
                TT(out=mm3, in0=me3, in1=x3[:, 20:21, :], op=mx)
                if t >= 2:
                    vector.wait_ge(sp, t - 1)   # matmuls(t-2) freed h slot
                TT(
                    out=h3,
                    in0=x3,
                    in1=mm3.to_broadcast((P, C, k)),
                    op=mybir.AluOpType.is_ge,
                ).then_inc(sv, 1)  # sv = t + 1
            vector.wait_ge(sp, NT)
            nc.vector.tensor_copy(ot[:], cm_psum[:]).then_inc(sv, 1)

        @block.tensor
        def _(tensor):
            for t in range(NT):
                kt = KT[t]
                yt = ys[t % 2][:]
                h3 = hs[t % 2][:, : C * kt].rearrange("p (c k) -> p c k", c=C)
                tensor.wait_ge(sv, t + 1)
                tensor.wait_ge(sys_[t % 2], arr(t))
                for k in range(kt):
                    j = k % 4  # column tile lane
                    mmu = nc.tensor.matmul(
                        out=cm_psum[:][32 * j : 32 * j + C, :],
                        lhsT=yt[:, k * C : (k + 1) * C],
                        rhs=h3[:, :, k : k + 1],
                        start=(t == 0 and k == j),
                        stop=(t == NT - 1 and k == kt - 4 + j),
                        tile_position=(0, 32 * j),
                        skip_group_check=True,
                    )
                mmu.then_inc(sp, 1)

    return nc


def _get_nc():
    global _CACHED_NC
    if _CACHED_NC is None:
        _CACHED_NC = build_nc()
    return _CACHED_NC


def make_in_maps(input, target):
    inp = np.asarray(input, dtype=np.float32)
    tgt = np.asarray(target, dtype=np.float32)
    in_maps = []
    NPP = N // P   # 2048 pixels per partition
    for b in range(B):
        # class-outer per-tile blocks, concatenated: [C, NPP] per partition
        xc = inp[b].reshape(C, P, NPP).transpose(1, 0, 2)   # [P, C, NPP]
        yp = tgt[b].reshape(C, P, NPP).transpose(1, 2, 0)   # [P, NPP, C] pix-major
        xflat = np.empty((P, C * NPP), dtype=np.float32)
        for t in range(NT):
            k0, k1 = KOFF[t], KOFF[t] + KT[t]
            xflat[:, C * k0 : C * k1] = xc[:, :, k0:k1].reshape(P, C * KT[t])
        in_maps.append(
            {
                "x": xflat.astype(X_NP_DT),
                "y": np.ascontiguousarray(yp).astype(Y_NP_DT).reshape(P, C * NPP),
            }
        )
    return in_maps


def postprocess(outs):
    acc = np.stack([np.asarray(o, dtype=np.float64) for o in outs])  # [B, P, C]
    raw = sum(acc[:, 32 * j : 32 * j + C, :] for j in range(4))
    cm = raw / (raw.sum(axis=2, keepdims=True) + 1e-30)
    return cm.mean(axis=0).astype(np.float32)


def kernel(input, target):
    nc = _get_nc()
    in_maps = make_in_maps(input, target)
    res = run_bass_kernel_spmd(nc, in_maps, list(range(N_CORES)))
    return postprocess([r["out"] for r in res.results])


# revision 23
# speedup vs baseline: 1.4710x; 1.0379x over previous
"""Confusion-matrix kernel for Trainium2 - fp8 inputs, non-uniform tiles.

Per batch b (one per core):
    pred[n]  = argmax_c input[b, c, n]
    raw[i, j] = sum_n target[b, i, n] * (pred[n] == j)
Host: cm_b = raw / rowsum(raw); out = mean_b cm_b.

x and y ship as fp8e4m3 (11 MB/core total).  The ACT engine upconverts each
x tile to fp16 (xf) so the DVE max-tree and is_ge run in 2x perf mode on
class-outer [P, C, Kt] tiles.  Tiles are non-uniform [128, 256, 512, 512,
512, 128] pixels/partition: small head tiles hide the ACT conversion latency
at pipeline fill, the small tail tile shrinks the serial is_ge+matmul tail.

Matmul: per pixel column k, on PE column-tile k%4 (128x32 column tiling ->
four concurrent ldweights+matmul lanes): lhsT = y[k] (fp8, 21 contiguous
bytes/partition, pixel-major), rhs = h3[:, :, k] (strided class-outer fp16),
each lane accumulating its own [21, 21] PSUM block at partitions 32j..
Host sums the 4 lane blocks, row-normalizes (absorbs fp8 argmax-tie double
counting), and means over batch.

Pipeline (2 buffers each for x8/xf/h/y):
    SP   : x loads even t; out store
    Pool : x loads odd t (first gated on x0 arrival)
    ACT  : y load issues + fp8->fp16 x conversions
    DVE  : max-tree -> is_ge(t)
    PE   : pack-matmuls(t)
"""

from contextlib import ExitStack

import ml_dtypes
import numpy as np

import concourse.bass as bass
import concourse.mybir as mybir
from concourse.bass_utils import run_bass_kernel_spmd

B, C, H, W = 8, 21, 512, 512
N = H * W
P = 128
KT = [128, 256, 512, 512, 512, 128]   # pixels/partition per tile
NT = len(KT)
assert sum(KT) == N // P
KOFF = [sum(KT[:t]) for t in range(NT)]   # flat pixel offsets
KMAX = max(KT)

N_CORES = 8

X_NP_DT = ml_dtypes.float8_e4m3
X_BIR_DT = mybir.dt.float8e4
Y_NP_DT = ml_dtypes.float8_e4m3
Y_BIR_DT = mybir.dt.float8e4

_CACHED_NC = None


def build_nc():
    nc = bass.Bass()
    # flat class-outer x / pixel-major y: per tile a contiguous [P, C*Kt] block
    x = nc.declare_dram_parameter("x", [P, C * (N // P)], X_BIR_DT, isOutput=False)
    y = nc.declare_dram_parameter("y", [P, C * (N // P)], Y_BIR_DT, isOutput=False)
    out = nc.declare_dram_parameter("out", [P, C], mybir.dt.float32, isOutput=True)

    with ExitStack() as ctx:
        x8 = [
            ctx.enter_context(nc.sbuf_tensor(f"x8b{i}", [P, C * KMAX], X_BIR_DT))
            for i in range(3)
        ]
        xf = [
            ctx.enter_context(nc.sbuf_tensor(f"xfb{i}", [P, C * KMAX], mybir.dt.float16))
            for i in range(2)
        ]
        ys = [
            ctx.enter_context(nc.sbuf_tensor(f"ysb{i}", [P, C * KMAX], Y_BIR_DT))
            for i in range(2)
        ]
        hs = [
            ctx.enter_context(nc.sbuf_tensor(f"hsb{i}", [P, C * KMAX], mybir.dt.float16))
            for i in range(3)
        ]
        ma = ctx.enter_context(nc.sbuf_tensor("ma", [P, 10 * KMAX], mybir.dt.float16))
        mb = ctx.enter_context(nc.sbuf_tensor("mb", [P, 5 * KMAX], mybir.dt.float16))
        mc = ctx.enter_context(nc.sbuf_tensor("mc", [P, 2 * KMAX], mybir.dt.float16))
        md = ctx.enter_context(nc.sbuf_tensor("md", [P, KMAX], mybir.dt.float16))
        me = ctx.enter_context(nc.sbuf_tensor("me", [P, KMAX], mybir.dt.float16))
        mm = ctx.enter_context(nc.sbuf_tensor("mm", [P, KMAX], mybir.dt.float16))
        ot = ctx.enter_context(nc.sbuf_tensor("otsb", [P, C], mybir.dt.float32))
        cm_psum = ctx.enter_context(nc.psum_tensor("cmps", [P, C], mybir.dt.float32))

        block = ctx.enter_context(nc.Block())
        sxs = [ctx.enter_context(nc.semaphore(f"sx{i}")) for i in range(3)]
        sys_ = [ctx.enter_context(nc.semaphore(f"sy{i}")) for i in range(2)]
        sf = ctx.enter_context(nc.semaphore("sf"))   # conversions done
        sv = ctx.enter_context(nc.semaphore("sv"))   # DVE tiles done
        sp = ctx.enter_context(nc.semaphore("sp"))   # PE tiles done
        so = ctx.enter_context(nc.semaphore("so"))

        def xin(t):
            return x[:, C * KOFF[t] : C * (KOFF[t] + KT[t])]

        def yin(t):
            return y[:, C * KOFF[t] : C * (KOFF[t] + KT[t])]

        # arrival count per buffer slot after tile t's DMA (inc 16 each)
        def arr(t):
            return 16 * (t // 2 + 1)

        def arr3(t):
            return 16 * (t // 3 + 1)

        def xdma(eng, t):
            # x8 slot t%3 freed once conv(t-3) consumed it
            if t >= 3:
                eng.wait_ge(sf, t - 2)
            eng.dma_start(out=x8[t % 3][:, : C * KT[t]], in_=xin(t)).then_inc(
                sxs[t % 3], 16
            )

        @block.sync
        def _(sync):
            xdma(sync, 0)
            sync.wait_ge(sxs[0], 16)      # x0 first and alone
            xdma(sync, 2)
            xdma(sync, 4)
            sync.wait_ge(sv, NT + 1)
            sync.dma_start(out=out[:], in_=ot[:]).then_inc(so, 16)
            sync.wait_ge(so, 16)

        def ydma(eng, t):
            if t >= 2:
                eng.wait_ge(sp, t - 1)    # matmuls(t-2) freed y slot
            eng.dma_start(out=ys[t % 2][:, : C * KT[t]], in_=yin(t)).then_inc(
                sys_[t % 2], 16
            )

        @block.gpsimd
        def _(gp):
            # x odd tiles + late y tiles.  Waits are ordered so no issue
            # blocks an earlier-needed one (sp/sf thresholds are increasing).
            gp.wait_ge(sxs[0], 16)        # x0 first and alone
            xdma(gp, 1)
            xdma(gp, 3)
            ydma(gp, 2)
            xdma(gp, 5)
            for t in range(3, NT):
                ydma(gp, t)

        @block.scalar
        def _(scalar):
            # y0/y1 issues (quick), then the fp8->fp16 conversion chain
            scalar.wait_ge(sxs[0], 16)    # let x0 use the DMA engines alone
            ydma(scalar, 0)
            ydma(scalar, 1)
            for t in range(NT):
                scalar.wait_ge(sxs[t % 3], arr3(t))
                if t >= 2:
                    scalar.wait_ge(sv, t - 1)  # DVE(t-2) done with xf slot
                nc.scalar.activation(
                    out=xf[t % 2][:, : C * KT[t]],
                    in_=x8[t % 3][:, : C * KT[t]],
                    func=mybir.ActivationFunctionType.Copy,
                ).then_inc(sf, 1)  # sf = t + 1

        @block.vector
        def _(vector):
            TT = nc.vector.tensor_tensor
            mx = mybir.AluOpType.max
            for t in range(NT):
                k = KT[t]
                x3 = xf[t % 2][:, : C * k].rearrange("p (c k) -> p c k", c=C)
                h3 = hs[t % 3][:, : C * k].rearrange("p (c k) -> p c k", c=C)
                ma3 = ma[:, : 10 * k].rearrange("p (c k) -> p c k", c=10)
                mb3 = mb[:, : 5 * k].rearrange("p (c k) -> p c k", c=5)
                mc3 = mc[:, : 2 * k].rearrange("p (c k) -> p c k", c=2)
                md3 = md[:, :k].unsqueeze(1)
                me3 = me[:, :k].unsqueeze(1)
                mm3 = mm[:, :k].unsqueeze(1)
                vector.wait_ge(sf, t + 1)
                TT(out=ma3, in0=x3[:, 0:10, :], in1=x3[:, 10:20, :], op=mx)
                TT(out=mb3, in0=ma3[:, 0:5, :], in1=ma3[:, 5:10, :], op=mx)
                TT(out=mc3, in0=mb3[:, 0:2, :], in1=mb3[:, 2:4, :], op=mx)
                TT(out=md3, in0=mc3[:, 0:1, :], in1=mc3[:, 1:2, :], op=mx)
                TT(out=me3, in0=md3, in1=mb3[:, 4:5, :], op=mx)
                TT(out=mm3, in0=me3, in1=x3[:, 20:21, :], op=mx)
                if t >= 3:
                    vector.wait_ge(sp, t - 2)   # matmuls(t-3) freed h slot
                TT(
                    out=h3,
                    in0=x3,
                    in1=mm3.to_broadcast((P, C, k)),
                    op=mybir.AluOpType.is_ge,
                ).then_inc(sv, 1)  # sv = t + 1
            vector.wait_ge(sp, NT)
            nc.vector.tensor_copy(ot[:], cm_psum[:]).then_inc(sv, 1)

        @block.tensor
        def _(tensor):
            for t in range(NT):
                kt = KT[t]
                yt = ys[t % 2][:]
                h3 = hs[t % 3][:, : C * kt].rearrange("p (c k) -> p c k", c=C)
                tensor.wait_ge(sv, t + 1)
                tensor.wait_ge(sys_[t % 2], arr(t))
                for k in range(kt):
                    j = k % 4  # column tile lane
                    mmu = nc.tensor.matmul(
                        out=cm_psum[:][32 * j : 32 * j + C, :],
                        lhsT=yt[:, k * C : (k + 1) * C],
                        rhs=h3[:, :, k : k + 1],
                        start=(t == 0 and k == j),
                        stop=(t == NT - 1 and k == kt - 4 + j),
                        tile_position=(0, 32 * j),
                        skip_group_check=True,
                    )
                mmu.then_inc(sp, 1)

    return nc


def _get_nc():
    global _CACHED_NC
    if _CACHED_NC is None:
        _CACHED_NC = build_nc()
    return _CACHED_NC


def make_in_maps(input, target):
    inp = np.asarray(input, dtype=np.float32)
    tgt = np.asarray(target, dtype=np.float32)
    in_maps = []
    NPP = N // P   # 2048 pixels per partition
    for b in range(B):
        # class-outer per-tile blocks, concatenated: [C, NPP] per partition
        xc = inp[b].reshape(C, P, NPP).transpose(1, 0, 2)   # [P, C, NPP]
        yp = tgt[b].reshape(C, P, NPP).transpose(1, 2, 0)   # [P, NPP, C] pix-major
        xflat = np.empty((P, C * NPP), dtype=np.float32)
        for t in range(NT):
            k0, k1 = KOFF[t], KOFF[t] + KT[t]
            xflat[:, C * k0 : C * k1] = xc[:, :, k0:k1].reshape(P, C * KT[t])
        in_maps.append(
            {
                "x": xflat.astype(X_NP_DT),
                "y": np.ascontiguousarray(yp).astype(Y_NP_DT).reshape(P, C * NPP),
            }
        )
    return in_maps


def postprocess(outs):
    acc = np.stack([np.asarray(o, dtype=np.float64) for o in outs])  # [B, P, C]
    raw = sum(acc[:, 32 * j : 32 * j + C, :] for j in range(4))
    cm = raw / (raw.sum(axis=2, keepdims=True) + 1e-30)
    return cm.mean(axis=0).astype(np.float32)


def kernel(input, target):
    nc = _get_nc()
    in_maps = make_in_maps(input, target)
    res = run_bass_kernel_spmd(nc, in_maps, list(range(N_CORES)))
    return postprocess([r["out"] for r in res.results])


# revision 29
# speedup vs baseline: 1.8527x; 1.2595x over previous
"""Confusion-matrix kernel for Trainium2 - fp8 inputs, non-uniform tiles.

Per batch b (one per core):
    pred[n]  = argmax_c input[b, c, n]
    raw[i, j] = sum_n target[b, i, n] * (pred[n] == j)
Host: cm_b = raw / rowsum(raw); out = mean_b cm_b.

x and y ship as fp8e4m3 (11 MB/core total).  The ACT engine upconverts each
x tile to fp16 (xf) so the DVE max-tree and is_ge run in 2x perf mode on
class-outer [P, C, Kt] tiles.  Tiles are non-uniform [128, 256, 512, 512,
512, 128] pixels/partition: small head tiles hide the ACT conversion latency
at pipeline fill, the small tail tile shrinks the serial is_ge+matmul tail.

Matmul: 2-pixel packs on 128x64 column tiling (2 concurrent ldweights+matmul
lanes, lane = pack%2, PSUM partitions 64j..64j+42).  lhsT = interleaved y
pair (fp8, 42 contiguous bytes/partition, col m = 2i+a for pack pixel a,
class i), rhs = h3[:, :, 2q:2q+2] (class-outer fp16, cols n = 2c+a -> pairs
of adjacent bytes).  Each pack instruction replaces two single-pixel
matmuls: half the PE instruction stream (the 64B-per-instruction fetch from
HBM stalls the PE ~1.7us every 16KB page).  out[2i+a, 2j+b]: the a==b
diagonal sub-blocks hold the confusion matrix; host sums them across the 2
pack positions and 2 lanes, row-normalizes (absorbs fp8 argmax-tie double
counting), and means over batch.

Pipeline (2 buffers each for x8/xf/h/y):
    SP   : x loads even t; out store
    Pool : x loads odd t (first gated on x0 arrival)
    ACT  : y load issues + fp8->fp16 x conversions
    DVE  : max-tree -> is_ge(t)
    PE   : pack-matmuls(t)
"""

from contextlib import ExitStack

import ml_dtypes
import numpy as np

import concourse.bass as bass
import concourse.mybir as mybir
from concourse.bass_utils import run_bass_kernel_spmd

B, C, H, W = 8, 21, 512, 512
N = H * W
P = 128
KT = [128, 256, 512, 512, 512, 128]   # pixels/partition per tile
NT = len(KT)
assert sum(KT) == N // P
KOFF = [sum(KT[:t]) for t in range(NT)]   # flat pixel offsets
KMAX = max(KT)

N_CORES = 8

X_NP_DT = ml_dtypes.float8_e4m3
X_BIR_DT = mybir.dt.float8e4
Y_NP_DT = ml_dtypes.float8_e4m3
Y_BIR_DT = mybir.dt.float8e4

_CACHED_NC = None


def build_nc():
    nc = bass.Bass()
    # flat class-outer x / pixel-major y: per tile a contiguous [P, C*Kt] block
    x = nc.declare_dram_parameter("x", [P, C * (N // P)], X_BIR_DT, isOutput=False)
    y = nc.declare_dram_parameter("y", [P, C * (N // P)], Y_BIR_DT, isOutput=False)
    out = nc.declare_dram_parameter("out", [P, 2 * C], mybir.dt.float32, isOutput=True)

    with ExitStack() as ctx:
        x8 = [
            ctx.enter_context(nc.sbuf_tensor(f"x8b{i}", [P, C * KMAX], X_BIR_DT))
            for i in range(3)
        ]
        xf = [
            ctx.enter_context(nc.sbuf_tensor(f"xfb{i}", [P, C * KMAX], mybir.dt.float16))
            for i in range(2)
        ]
        ys = [
            ctx.enter_context(nc.sbuf_tensor(f"ysb{i}", [P, C * KMAX], Y_BIR_DT))
            for i in range(2)
        ]
        hs = [
            ctx.enter_context(nc.sbuf_tensor(f"hsb{i}", [P, C * KMAX], mybir.dt.float16))
            for i in range(3)
        ]
        ma = ctx.enter_context(nc.sbuf_tensor("ma", [P, 10 * KMAX], mybir.dt.float16))
        mb = ctx.enter_context(nc.sbuf_tensor("mb", [P, 5 * KMAX], mybir.dt.float16))
        mc = ctx.enter_context(nc.sbuf_tensor("mc", [P, 2 * KMAX], mybir.dt.float16))
        md = ctx.enter_context(nc.sbuf_tensor("md", [P, KMAX], mybir.dt.float16))
        me = ctx.enter_context(nc.sbuf_tensor("me", [P, KMAX], mybir.dt.float16))
        mm = ctx.enter_context(nc.sbuf_tensor("mm", [P, KMAX], mybir.dt.float16))
        ot = ctx.enter_context(nc.sbuf_tensor("otsb", [P, 2 * C], mybir.dt.float32))
        cm_psum = ctx.enter_context(nc.psum_tensor("cmps", [P, 2 * C], mybir.dt.float32))

        block = ctx.enter_context(nc.Block())
        sxs = [ctx.enter_context(nc.semaphore(f"sx{i}")) for i in range(3)]
        sys_ = [ctx.enter_context(nc.semaphore(f"sy{i}")) for i in range(2)]
        sf = ctx.enter_context(nc.semaphore("sf"))   # conversions done
        sv = ctx.enter_context(nc.semaphore("sv"))   # DVE tiles done
        sp = ctx.enter_context(nc.semaphore("sp"))   # PE tiles done
        so = ctx.enter_context(nc.semaphore("so"))

        def xin(t):
            return x[:, C * KOFF[t] : C * (KOFF[t] + KT[t])]

        def yin(t):
            return y[:, C * KOFF[t] : C * (KOFF[t] + KT[t])]

        # arrival count per buffer slot after tile t's DMA (inc 16 each)
        def arr(t):
            return 16 * (t // 2 + 1)

        def arr3(t):
            return 16 * (t // 3 + 1)

        def xdma(eng, t):
            # x8 slot t%3 freed once conv(t-3) consumed it
            if t >= 3:
                eng.wait_ge(sf, t - 2)
            eng.dma_start(out=x8[t % 3][:, : C * KT[t]], in_=xin(t)).then_inc(
                sxs[t % 3], 16
            )

        @block.sync
        def _(sync):
            xdma(sync, 0)
            sync.wait_ge(sxs[0], 16)      # x0 first and alone
            xdma(sync, 2)
            xdma(sync, 4)
            sync.wait_ge(sv, NT + 1)
            sync.dma_start(out=out[:], in_=ot[:]).then_inc(so, 16)
            sync.wait_ge(so, 16)

        def ydma(eng, t):
            if t >= 2:
                eng.wait_ge(sp, t - 1)    # matmuls(t-2) freed y slot
            eng.dma_start(out=ys[t % 2][:, : C * KT[t]], in_=yin(t)).then_inc(
                sys_[t % 2], 16
            )

        @block.gpsimd
        def _(gp):
            # x odd tiles + late y tiles.  Waits are ordered so no issue
            # blocks an earlier-needed one (sp/sf thresholds are increasing).
            gp.wait_ge(sxs[0], 16)        # x0 first and alone
            xdma(gp, 1)
            xdma(gp, 3)
            ydma(gp, 2)
            xdma(gp, 5)
            for t in range(3, NT):
                ydma(gp, t)

        @block.scalar
        def _(scalar):
            # y0/y1 issues (quick), then the fp8->fp16 conversion chain
            scalar.wait_ge(sxs[0], 16)    # let x0 use the DMA engines alone
            ydma(scalar, 0)
            ydma(scalar, 1)
            for t in range(NT):
                scalar.wait_ge(sxs[t % 3], arr3(t))
                if t >= 2:
                    scalar.wait_ge(sv, t - 1)  # DVE(t-2) done with xf slot
                nc.scalar.activation(
                    out=xf[t % 2][:, : C * KT[t]],
                    in_=x8[t % 3][:, : C * KT[t]],
                    func=mybir.ActivationFunctionType.Copy,
                ).then_inc(sf, 1)  # sf = t + 1

        @block.vector
        def _(vector):
            TT = nc.vector.tensor_tensor
            mx = mybir.AluOpType.max
            for t in range(NT):
                k = KT[t]
                x3 = xf[t % 2][:, : C * k].rearrange("p (c k) -> p c k", c=C)
                h3 = hs[t % 3][:, : C * k].rearrange("p (c k) -> p c k", c=C)
                ma3 = ma[:, : 10 * k].rearrange("p (c k) -> p c k", c=10)
                mb3 = mb[:, : 5 * k].rearrange("p (c k) -> p c k", c=5)
                mc3 = mc[:, : 2 * k].rearrange("p (c k) -> p c k", c=2)
                md3 = md[:, :k].unsqueeze(1)
                me3 = me[:, :k].unsqueeze(1)
                mm3 = mm[:, :k].unsqueeze(1)
                vector.wait_ge(sf, t + 1)
                TT(out=ma3, in0=x3[:, 0:10, :], in1=x3[:, 10:20, :], op=mx)
                TT(out=mb3, in0=ma3[:, 0:5, :], in1=ma3[:, 5:10, :], op=mx)
                TT(out=mc3, in0=mb3[:, 0:2, :], in1=mb3[:, 2:4, :], op=mx)
                TT(out=md3, in0=mc3[:, 0:1, :], in1=mc3[:, 1:2, :], op=mx)
                TT(out=me3, in0=md3, in1=mb3[:, 4:5, :], op=mx)
                TT(out=mm3, in0=me3, in1=x3[:, 20:21, :], op=mx)
                if t >= 3:
                    vector.wait_ge(sp, t - 2)   # matmuls(t-3) freed h slot
                TT(
                    out=h3,
                    in0=x3,
                    in1=mm3.to_broadcast((P, C, k)),
                    op=mybir.AluOpType.is_ge,
                ).then_inc(sv, 1)  # sv = t + 1
            vector.wait_ge(sp, NT)
            nc.vector.tensor_copy(ot[:], cm_psum[:]).then_inc(sv, 1)

        @block.tensor
        def _(tensor):
            for t in range(NT):
                kt = KT[t]
                yt = ys[t % 2][:]
                h3 = hs[t % 3][:, : C * kt].rearrange("p (c k) -> p c k", c=C)
                tensor.wait_ge(sv, t + 1)
                tensor.wait_ge(sys_[t % 2], arr(t))
                for q in range(kt // 2):
                    j = q % 2  # column tile lane
                    mmu = nc.tensor.matmul(
                        out=cm_psum[:][64 * j : 64 * j + 2 * C, :],
                        lhsT=yt[:, q * 2 * C : (q + 1) * 2 * C],
                        rhs=h3[:, :, 2 * q : 2 * q + 2],
                        start=(t == 0 and q == j),
                        stop=(t == NT - 1 and q == kt // 2 - 2 + j),
                        tile_position=(0, 64 * j),
                        skip_group_check=True,
                    )
                mmu.then_inc(sp, 1)

    return nc


def _get_nc():
    global _CACHED_NC
    if _CACHED_NC is None:
        _CACHED_NC = build_nc()
    return _CACHED_NC


def make_in_maps(input, target):
    inp = np.asarray(input, dtype=np.float32)
    tgt = np.asarray(target, dtype=np.float32)
    in_maps = []
    NPP = N // P   # 2048 pixels per partition
    for b in range(B):
        # class-outer per-tile blocks, concatenated: [C, NPP] per partition
        xc = inp[b].reshape(C, P, NPP).transpose(1, 0, 2)   # [P, C, NPP]
        yp = tgt[b].reshape(C, P, NPP).transpose(1, 2, 0)   # [P, NPP, C] pix-major
        xflat = np.empty((P, C * NPP), dtype=np.float32)
        for t in range(NT):
            k0, k1 = KOFF[t], KOFF[t] + KT[t]
            xflat[:, C * k0 : C * k1] = xc[:, :, k0:k1].reshape(P, C * KT[t])
        # interleave pixel pairs: pack q cols m = 2i+a = y[pixel 2q+a, class i]
        y2 = yp.reshape(P, NPP // 2, 2, C).transpose(0, 1, 3, 2)
        in_maps.append(
            {
                "x": xflat.astype(X_NP_DT),
                "y": np.ascontiguousarray(y2).astype(Y_NP_DT).reshape(P, C * NPP),
            }
        )
    return in_maps


def postprocess(outs):
    acc = np.stack([np.asarray(o, dtype=np.float64) for o in outs])  # [B, P, 2C]
    raw = 0
    for j in range(2):  # column tile lanes
        blk = acc[:, 64 * j : 64 * j + 2 * C, :].reshape(-1, C, 2, C, 2)
        raw = raw + blk[:, :, 0, :, 0] + blk[:, :, 1, :, 1]
    cm = raw / (raw.sum(axis=2, keepdims=True) + 1e-30)
    return cm.mean(axis=0).astype(np.float32)


def kernel(input, target):
    nc = _get_nc()
    in_maps = make_in_maps(input, target)
    res = run_bass_kernel_spmd(nc, in_maps, list(range(N_CORES)))
    return postprocess([r["out"] for r in res.results])


# revision 32
# speedup vs baseline: 1.8714x; 1.0101x over previous
"""Confusion-matrix kernel for Trainium2 - fp8 inputs, non-uniform tiles.

Per batch b (one per core):
    pred[n]  = argmax_c input[b, c, n]
    raw[i, j] = sum_n target[b, i, n] * (pred[n] == j)
Host: cm_b = raw / rowsum(raw); out = mean_b cm_b.

x and y ship as fp8e4m3 (11 MB/core total).  The ACT engine upconverts each
x tile to fp16 (xf) so the DVE max-tree and is_ge run in 2x perf mode on
class-outer [P, C, Kt] tiles.  Tile sizes ramp up then down ([128, 192, 384,
512, 512, 192, 128] pixels/partition): the ACT conversion chain stays ahead
of the DVE during pipeline fill, and the small tail tile shrinks the serial
is_ge -> matmul -> store ending.

Matmul: 2-pixel packs on 128x64 column tiling (2 concurrent ldweights+matmul
lanes, lane = pack%2, PSUM partitions 64j..64j+42).  lhsT = interleaved y
pair (fp8, 42 contiguous bytes/partition, col m = 2i+a for pack pixel a,
class i), rhs = h3[:, :, 2q:2q+2] (class-outer fp16, cols n = 2c+a -> pairs
of adjacent bytes).  Each pack instruction replaces two single-pixel
matmuls: half the PE instruction stream (the 64B-per-instruction fetch from
HBM stalls the PE ~1.7us every 16KB page).  out[2i+a, 2j+b]: the a==b
diagonal sub-blocks hold the confusion matrix; host sums them across the 2
pack positions and 2 lanes, row-normalizes (absorbs fp8 argmax-tie double
counting), and means over batch.

Pipeline (2 buffers each for x8/xf/h/y):
    SP   : x loads even t; out store
    Pool : x loads odd t (first gated on x0 arrival)
    ACT  : y load issues + fp8->fp16 x conversions
    DVE  : max-tree -> is_ge(t)
    PE   : pack-matmuls(t)
"""

from contextlib import ExitStack

import ml_dtypes
import numpy as np

import concourse.bass as bass
import concourse.mybir as mybir
from concourse.bass_utils import run_bass_kernel_spmd

B, C, H, W = 8, 21, 512, 512
N = H * W
P = 128
KT = [128, 192, 384, 512, 512, 192, 128]   # pixels/partition per tile
NT = len(KT)
assert sum(KT) == N // P
KOFF = [sum(KT[:t]) for t in range(NT)]   # flat pixel offsets
KMAX = max(KT)

N_CORES = 8

X_NP_DT = ml_dtypes.float8_e4m3
X_BIR_DT = mybir.dt.float8e4
Y_NP_DT = ml_dtypes.float8_e4m3
Y_BIR_DT = mybir.dt.float8e4

_CACHED_NC = None


def build_nc():
    nc = bass.Bass()
    # flat class-outer x / pixel-major y: per tile a contiguous [P, C*Kt] block
    x = nc.declare_dram_parameter("x", [P, C * (N // P)], X_BIR_DT, isOutput=False)
    y = nc.declare_dram_parameter("y", [P, C * (N // P)], Y_BIR_DT, isOutput=False)
    out = nc.declare_dram_parameter("out", [P, 2 * C], mybir.dt.float32, isOutput=True)

    with ExitStack() as ctx:
        x8 = [
            ctx.enter_context(nc.sbuf_tensor(f"x8b{i}", [P, C * KMAX], X_BIR_DT))
            for i in range(3)
        ]
        xf = [
            ctx.enter_context(nc.sbuf_tensor(f"xfb{i}", [P, C * KMAX], mybir.dt.float16))
            for i in range(2)
        ]
        ys = [
            ctx.enter_context(nc.sbuf_tensor(f"ysb{i}", [P, C * KMAX], Y_BIR_DT))
            for i in range(2)
        ]
        hs = [
            ctx.enter_context(nc.sbuf_tensor(f"hsb{i}", [P, C * KMAX], mybir.dt.float16))
            for i in range(3)
        ]
        ma = ctx.enter_context(nc.sbuf_tensor("ma", [P, 10 * KMAX], mybir.dt.float16))
        mb = ctx.enter_context(nc.sbuf_tensor("mb", [P, 5 * KMAX], mybir.dt.float16))
        mc = ctx.enter_context(nc.sbuf_tensor("mc", [P, 2 * KMAX], mybir.dt.float16))
        md = ctx.enter_context(nc.sbuf_tensor("md", [P, KMAX], mybir.dt.float16))
        me = ctx.enter_context(nc.sbuf_tensor("me", [P, KMAX], mybir.dt.float16))
        mm = ctx.enter_context(nc.sbuf_tensor("mm", [P, KMAX], mybir.dt.float16))
        ot = ctx.enter_context(nc.sbuf_tensor("otsb", [P, 2 * C], mybir.dt.float32))
        cm_psum = ctx.enter_context(nc.psum_tensor("cmps", [P, 2 * C], mybir.dt.float32))

        block = ctx.enter_context(nc.Block())
        sxs = [ctx.enter_context(nc.semaphore(f"sx{i}")) for i in range(3)]
        sys_ = [ctx.enter_context(nc.semaphore(f"sy{i}")) for i in range(2)]
        sf = ctx.enter_context(nc.semaphore("sf"))   # conversions done
        sv = ctx.enter_context(nc.semaphore("sv"))   # DVE tiles done
        sp = ctx.enter_context(nc.semaphore("sp"))   # PE tiles done
        so = ctx.enter_context(nc.semaphore("so"))

        def xin(t):
            return x[:, C * KOFF[t] : C * (KOFF[t] + KT[t])]

        def yin(t):
            return y[:, C * KOFF[t] : C * (KOFF[t] + KT[t])]

        # arrival count per buffer slot after tile t's DMA (inc 16 each)
        def arr(t):
            return 16 * (t // 2 + 1)

        def arr3(t):
            return 16 * (t // 3 + 1)

        def xdma(eng, t):
            # x8 slot t%3 freed once conv(t-3) consumed it
            if t >= 3:
                eng.wait_ge(sf, t - 2)
            eng.dma_start(out=x8[t % 3][:, : C * KT[t]], in_=xin(t)).then_inc(
                sxs[t % 3], 16
            )

        @block.sync
        def _(sync):
            xdma(sync, 0)
            sync.wait_ge(sxs[0], 16)      # x0 first and alone
            for t in range(2, NT, 2):
                xdma(sync, t)
            sync.wait_ge(sv, NT + 1)
            sync.dma_start(out=out[:], in_=ot[:]).then_inc(so, 16)
            sync.wait_ge(so, 16)

        def ydma(eng, t):
            if t >= 2:
                eng.wait_ge(sp, t - 1)    # matmuls(t-2) freed y slot
            eng.dma_start(out=ys[t % 2][:, : C * KT[t]], in_=yin(t)).then_inc(
                sys_[t % 2], 16
            )

        @block.gpsimd
        def _(gp):
            # x odd tiles + late y tiles.  Waits are ordered so no issue
            # blocks an earlier-needed one (sp/sf thresholds are increasing).
            gp.wait_ge(sxs[0], 16)        # x0 first and alone
            xdma(gp, 1)
            xdma(gp, 3)
            ydma(gp, 2)
            xdma(gp, 5)
            for t in range(3, NT):
                ydma(gp, t)

        @block.scalar
        def _(scalar):
            # y0/y1 issues (quick), then the fp8->fp16 conversion chain
            scalar.wait_ge(sxs[0], 16)    # let x0 use the DMA engines alone
            ydma(scalar, 0)
            ydma(scalar, 1)
            for t in range(NT):
                scalar.wait_ge(sxs[t % 3], arr3(t))
                if t >= 2:
                    scalar.wait_ge(sv, t - 1)  # DVE(t-2) done with xf slot
                nc.scalar.activation(
                    out=xf[t % 2][:, : C * KT[t]],
                    in_=x8[t % 3][:, : C * KT[t]],
                    func=mybir.ActivationFunctionType.Copy,
                ).then_inc(sf, 1)  # sf = t + 1

        @block.vector
        def _(vector):
            TT = nc.vector.tensor_tensor
            mx = mybir.AluOpType.max
            for t in range(NT):
                k = KT[t]
                x3 = xf[t % 2][:, : C * k].rearrange("p (c k) -> p c k", c=C)
                h3 = hs[t % 3][:, : C * k].rearrange("p (c k) -> p c k", c=C)
                ma3 = ma[:, : 10 * k].rearrange("p (c k) -> p c k", c=10)
                mb3 = mb[:, : 5 * k].rearrange("p (c k) -> p c k", c=5)
                mc3 = mc[:, : 2 * k].rearrange("p (c k) -> p c k", c=2)
                md3 = md[:, :k].unsqueeze(1)
                me3 = me[:, :k].unsqueeze(1)
                mm3 = mm[:, :k].unsqueeze(1)
                vector.wait_ge(sf, t + 1)
                TT(out=ma3, in0=x3[:, 0:10, :], in1=x3[:, 10:20, :], op=mx)
                TT(out=mb3, in0=ma3[:, 0:5, :], in1=ma3[:, 5:10, :], op=mx)
                TT(out=mc3, in0=mb3[:, 0:2, :], in1=mb3[:, 2:4, :], op=mx)
                TT(out=md3, in0=mc3[:, 0:1, :], in1=mc3[:, 1:2, :], op=mx)
                TT(out=me3, in0=md3, in1=mb3[:, 4:5, :], op=mx)
                TT(out=mm3, in0=me3, in1=x3[:, 20:21, :], op=mx)
                if t >= 3:
                    vector.wait_ge(sp, t - 2)   # matmuls(t-3) freed h slot
                TT(
                    out=h3,
                    in0=x3,
                    in1=mm3.to_broadcast((P, C, k)),
                    op=mybir.AluOpType.is_ge,
                ).then_inc(sv, 1)  # sv = t + 1
            vector.wait_ge(sp, NT)
            nc.vector.tensor_copy(ot[:], cm_psum[:]).then_inc(sv, 1)

        @block.tensor
        def _(tensor):
            for t in range(NT):
                kt = KT[t]
                yt = ys[t % 2][:]
                h3 = hs[t % 3][:, : C * kt].rearrange("p (c k) -> p c k", c=C)
                tensor.wait_ge(sv, t + 1)
                tensor.wait_ge(sys_[t % 2], arr(t))
                for q in range(kt // 2):
                    j = q % 2  # column tile lane
                    mmu = nc.tensor.matmul(
                        out=cm_psum[:][64 * j : 64 * j + 2 * C, :],
                        lhsT=yt[:, q * 2 * C : (q + 1) * 2 * C],
                        rhs=h3[:, :, 2 * q : 2 * q + 2],
                        start=(t == 0 and q == j),
                        stop=(t == NT - 1 and q == kt // 2 - 2 + j),
                        tile_position=(0, 64 * j),
                        skip_group_check=True,
                    )
                mmu.then_inc(sp, 1)

    return nc


def _get_nc():
    global _CACHED_NC
    if _CACHED_NC is None:
        _CACHED_NC = build_nc()
    return _CACHED_NC


def make_in_maps(input, target):
    inp = np.asarray(input, dtype=np.float32)
    tgt = np.asarray(target, dtype=np.float32)
    in_maps = []
    NPP = N // P   # 2048 pixels per partition
    for b in range(B):
        # class-outer per-tile blocks, concatenated: [C, NPP] per partition
        xc = inp[b].reshape(C, P, NPP).transpose(1, 0, 2)   # [P, C, NPP]
        yp = tgt[b].reshape(C, P, NPP).transpose(1, 2, 0)   # [P, NPP, C] pix-major
        xflat = np.empty((P, C * NPP), dtype=np.float32)
        for t in range(NT):
            k0, k1 = KOFF[t], KOFF[t] + KT[t]
            xflat[:, C * k0 : C * k1] = xc[:, :, k0:k1].reshape(P, C * KT[t])
        # interleave pixel pairs: pack q cols m = 2i+a = y[pixel 2q+a, class i]
        y2 = yp.reshape(P, NPP // 2, 2, C).transpose(0, 1, 3, 2)
        in_maps.append(
            {
                "x": xflat.astype(X_NP_DT),
                "y": np.ascontiguousarray(y2).astype(Y_NP_DT).reshape(P, C * NPP),
            }
        )
    return in_maps


def postprocess(outs):
    acc = np.stack([np.asarray(o, dtype=np.float64) for o in outs])  # [B, P, 2C]
    raw = 0
    for j in range(2):  # column tile lanes
        blk = acc[:, 64 * j : 64 * j + 2 * C, :].reshape(-1, C, 2, C, 2)
        raw = raw + blk[:, :, 0, :, 0] + blk[:, :, 1, :, 1]
    cm = raw / (raw.sum(axis=2, keepdims=True) + 1e-30)
    return cm.mean(axis=0).astype(np.float32)


def kernel(input, target):
    nc = _get_nc()
    in_maps = make_in_maps(input, target)
    res = run_bass_kernel_spmd(nc, in_maps, list(range(N_CORES)))
    return postprocess([r["out"] for r in res.results])
